# revision 23
# baseline (speedup 1.0000x reference)
"""v2: instruction-count-minimized kernel for the axon-tunneled trn2 backend.

The execution backend charges ~34us per compute instruction regardless of
size; DMA instructions are cheap. So: fewest, widest compute instructions.

Per core (32768 samples), per super-chunk SC of 4096 samples (all fp32):
  - xt [8, 4096] fm tile loaded by ONE strided gather DMA (col = global row)
  - L1/L2/L3: 8x [128,512]-moving matmuls each -> P [128, 4096] (8 psum banks)
  - bias+relu fused in ONE wide DVE op [128, 4096] per layer
  - L4/L5 heads: 8 matmuls each -> P[0:2, :] and P[32:34, :] (fm)
  - one [34, 4096] PSUM->SBUF copy, two cheap DMAs park heads in DRAM
Epilogue (whole core at once):
  - DMAs gather heads DRAM -> bm32 [32, (1024, 4)] (sample s at
    partition s%32) and x[:, 0:2] -> [32, (1024, 2)]
  - ~20 wide DVE/ACT ops compute the closed-form QP
  - DMAs scatter u back to out[g, 0:2]
"""

import os
import sys

import numpy as np

for _p in ("/opt/trn_rl_repo", os.path.expanduser("~/.axon_site/_ro/trn_rl_repo")):
    if os.path.isdir(_p) and _p not in sys.path:
        sys.path.append(_p)

import concourse.bacc as bacc
import concourse.mybir as mybir
import concourse.tile as tile
from concourse.bass_utils import run_bass_kernel_spmd

dt = mybir.dt
AF = mybir.ActivationFunctionType
ALU = mybir.AluOpType

N_CORES = 8
B_FULL, F, H1, H2, C = 262144, 8, 128, 128, 2
BS = B_FULL // N_CORES    # 32768 per core
P = 128
SC = 4096                 # samples per super-chunk
NSC = BS // SC            # 8
OBS_X, OBS_Y, RADIUS = 40.0, 15.0, 6.0
EPS = 1e-12

# fp32 const block [128, CW32]
C_LT1 = 0      # (w1*std).T in rows 0:8   [8, 128]
C_B1 = 128     # b1 + w1@mean             [128, 1]
C_B21 = 129    # b21                      [128, 1]
C_B22 = 130    # b22                      [128, 1]
C_S0 = 131     # scalar std[0] broadcast  [128, 1]
C_S1 = 132     # std[1]
C_M0 = 133     # mean[0] - OBS_X
C_M1 = 134     # mean[1] - OBS_Y
C_B31A = 135   # b31[0]
C_B31B = 136   # b31[1]
C_B32A = 137   # b32[0]
CW32 = 398
C_LT2 = 138    # w21.T [128, 128]
C_LT3 = 266    # w22.T [128, 128]
C_W31 = 394    # w31.T [128, 2]
C_W32 = 396    # w32.T [128, 2]



def build_program(repeat=1):
    nc = bacc.Bacc("TRN2", target_bir_lowering=False, debug=False,
                   num_devices=N_CORES)
    f32 = dt.float32

    NQ = BS // 32
    x_d = nc.dram_tensor("x", [BS, F], f32, kind="ExternalInput").ap()
    c32_d = nc.dram_tensor("c32", [P, CW32], f32, kind="ExternalInput").ap()
    out_d = nc.dram_tensor("out", [BS, C], f32, kind="ExternalOutput").ap()
    hb_d = nc.dram_tensor("hb", [4, BS], f32).ap()  # heads parked fm in DRAM

    with tile.TileContext(nc) as tc:
        with (
            tc.tile_pool(name="cst", bufs=1) as cstp,
            tc.tile_pool(name="xt", bufs=1) as xtp,
            tc.tile_pool(name="act", bufs=1) as actp,
            tc.tile_pool(name="hd", bufs=1) as hdp,
            tc.tile_pool(name="ep", bufs=1) as epp,
            tc.tile_pool(name="ps", bufs=1, space="PSUM") as psp,
        ):
            c32 = cstp.tile([P, CW32], f32)
            nc.sync.dma_start(c32[:], c32_d[:])

            lt1 = c32[0:F, C_LT1:C_LT1 + H1]
            b1 = c32[:, C_B1:C_B1 + 1]
            b21 = c32[:, C_B21:C_B21 + 1]
            b22 = c32[:, C_B22:C_B22 + 1]
            lt2 = c32[:, C_LT2:C_LT2 + H2]
            lt3 = c32[:, C_LT3:C_LT3 + H2]
            w31 = c32[:, C_W31:C_W31 + 2]
            w32 = c32[:, C_W32:C_W32 + 2]

            for _rep in range(repeat):
                for sc in range(NSC):
                    g0 = sc * SC
                    # fm gather: xt[f, c] = x[g0 + c, f]
                    xt = xtp.tile([F, SC], f32, tag="xt")
                    nc.sync.dma_start(
                        xt[:], x_d[g0:g0 + SC, :].rearrange("g f -> f g"))

                    Ppre = psp.tile([P, SC], f32, tag="P")  # all 8 banks
                    for b in range(8):
                        nc.tensor.matmul(Ppre[:, 512 * b:512 * (b + 1)],
                                         lt1, xt[:, 512 * b:512 * (b + 1)],
                                         start=True, stop=True)
                    h1 = actp.tile([P, SC], f32, tag="h1")
                    nc.vector.tensor_scalar(h1[:], Ppre[:], b1, 0.0,
                                            ALU.add, ALU.max)

                    Ppre2 = psp.tile([P, SC], f32, tag="P")
                    for b in range(8):
                        nc.tensor.matmul(Ppre2[:, 512 * b:512 * (b + 1)],
                                         lt2, h1[:, 512 * b:512 * (b + 1)],
                                         start=True, stop=True)
                    x21 = actp.tile([P, SC], f32, tag="x21")
                    nc.vector.tensor_scalar(x21[:], Ppre2[:], b21, 0.0,
                                            ALU.add, ALU.max)

                    Ppre3 = psp.tile([P, SC], f32, tag="P")
                    for b in range(8):
                        nc.tensor.matmul(Ppre3[:, 512 * b:512 * (b + 1)],
                                         lt3, h1[:, 512 * b:512 * (b + 1)],
                                         start=True, stop=True)
                    x22 = actp.tile([P, SC], f32, tag="x22")
                    nc.vector.tensor_scalar(x22[:], Ppre3[:], b22, 0.0,
                                            ALU.add, ALU.max)

                    PH = psp.tile([P, SC], f32, tag="P")
                    for b in range(8):
                        nc.tensor.matmul(PH[0:2, 512 * b:512 * (b + 1)],
                                         w31, x21[:, 512 * b:512 * (b + 1)],
                                         start=True, stop=True)
                    for b in range(8):
                        nc.tensor.matmul(PH[32:34, 512 * b:512 * (b + 1)],
                                         w32, x22[:, 512 * b:512 * (b + 1)],
                                         start=True, stop=True)
                    hs = hdp.tile([34, SC], f32, tag="hs")
                    nc.vector.tensor_copy(hs[:], PH[0:34, :])
                    nc.sync.dma_start(hb_d[0:2, g0:g0 + SC], hs[0:2, :])
                    nc.sync.dma_start(hb_d[2:4, g0:g0 + SC], hs[32:34, :])

                # ---- epilogue: whole core, batch-major-32 ----
                hbm = epp.tile([32, NQ, 4], f32, tag="hbm")
                # hbm[m, n, k] = hb[k, 32n + m]
                hbv = hb_d.rearrange("k (n m) -> k m n", m=32)
                for k in range(4):
                    nc.sync.dma_start(hbm[:, :, k], hbv[k])
                xb = epp.tile([32, NQ, 2], f32, tag="xb")
                xbv = x_d[:, 0:2].rearrange("(n m) c -> c m n", m=32)
                for k in range(2):
                    nc.sync.dma_start(xb[:, :, k], xbv[k])

                pp = epp.tile([32, NQ, 2], f32, tag="pp")
                d_t = epp.tile([32, NQ, 2], f32, tag="d_t")
                dsq = epp.tile([32, NQ, 2], f32, tag="dsq")
                dp = epp.tile([32, NQ, 2], f32, tag="dp")
                sp0 = epp.tile([32, NQ], f32, tag="sp0")
                sig0 = epp.tile([32, NQ], f32, tag="sig0")
                bar0 = epp.tile([32, NQ], f32, tag="bar0")
                bar = epp.tile([32, NQ], f32, tag="bar")
                v1 = epp.tile([32, NQ], f32, tag="v1")
                hb2 = epp.tile([32, NQ], f32, tag="hb2")
                viol2 = epp.tile([32, NQ], f32, tag="viol2")
                ggq = epp.tile([32, NQ], f32, tag="ggq")
                rec = epp.tile([32, NQ], f32, tag="rec")
                lam2 = epp.tile([32, NQ], f32, tag="lam2")
                u_t = epp.tile([32, NQ, 2], f32, tag="u_t")

                V = nc.vector
                s0c = c32[0:32, C_S0:C_S0 + 1]
                s1c = c32[0:32, C_S1:C_S1 + 1]
                m0c = c32[0:32, C_M0:C_M0 + 1]
                m1c = c32[0:32, C_M1:C_M1 + 1]
                b31a = c32[0:32, C_B31A:C_B31A + 1]
                b31b = c32[0:32, C_B31B:C_B31B + 1]
                b32a = c32[0:32, C_B32A:C_B32A + 1]

                # d = x01 * std01 + (mean01 - obs)
                V.tensor_scalar(d_t[:, :, 0], xb[:, :, 0], s0c, None, ALU.mult)
                V.tensor_scalar(d_t[:, :, 0], d_t[:, :, 0], m0c, None, ALU.add)
                V.tensor_scalar(d_t[:, :, 1], xb[:, :, 1], s1c, None, ALU.mult)
                V.tensor_scalar(d_t[:, :, 1], d_t[:, :, 1], m1c, None, ALU.add)
                V.tensor_tensor(dsq[:], d_t[:], d_t[:], ALU.mult)
                # p' = P + b31
                V.tensor_scalar(pp[:, :, 0], hbm[:, :, 0], b31a, None, ALU.add)
                V.tensor_scalar(pp[:, :, 1], hbm[:, :, 1], b31b, None, ALU.add)
                V.tensor_tensor(dp[:], d_t[:], pp[:], ALU.mult)
                # s'0 = S0 + b32[0]; sig0 = sigmoid(s'0)
                V.tensor_scalar(sp0[:], hbm[:, :, 2], b32a, None, ALU.add)
                nc.scalar.activation(sig0[:], sp0[:], AF.Sigmoid)
                V.tensor_tensor(bar0[:], dsq[:, :, 0], dsq[:, :, 1], ALU.add)
                V.tensor_tensor(v1[:], dp[:, :, 0], dp[:, :, 1], ALU.add)
                V.tensor_scalar(bar[:], bar0[:], 2.0, -2.0 * RADIUS * RADIUS,
                                ALU.mult, ALU.add)
                V.tensor_tensor(hb2[:], sig0[:], bar[:], ALU.mult)
                V.tensor_tensor(viol2[:], v1[:], hb2[:], ALU.subtract)
                # lam2 = relu(viol2) / (bar0 + eps/4)
                V.tensor_scalar(viol2[:], viol2[:], 0.0, None, ALU.max)
                V.tensor_scalar(ggq[:], bar0[:], EPS / 4.0, None, ALU.add)
                V.reciprocal(rec[:], ggq[:])
                V.tensor_tensor(lam2[:], viol2[:], rec[:], ALU.mult)
                # u = d * lam2 - p'
                V.tensor_tensor(u_t[:, :, 0], d_t[:, :, 0], lam2[:], ALU.mult)
                V.tensor_tensor(u_t[:, :, 1], d_t[:, :, 1], lam2[:], ALU.mult)
                V.tensor_tensor(u_t[:], u_t[:], pp[:], ALU.subtract)

                outv = out_d.rearrange("(n m) c -> c m n", m=32)
                for k in range(2):
                    nc.sync.dma_start(outv[k], u_t[:, :, k])

    nc.compile()
    return nc


def make_consts(mean, std, w1, b1, w21, b21, w22, b22, w31, b31, w32, b32):
    c32 = np.zeros((P, CW32), dtype=np.float32)
    c32[0:F, C_LT1:C_LT1 + H1] = (w1 * std[None, :]).T
    c32[:, C_B1] = b1 + w1 @ mean
    c32[:, C_B21] = b21
    c32[:, C_B22] = b22
    c32[:, C_S0] = std[0]
    c32[:, C_S1] = std[1]
    c32[:, C_M0] = mean[0] - OBS_X
    c32[:, C_M1] = mean[1] - OBS_Y
    c32[:, C_B31A] = b31[0]
    c32[:, C_B31B] = b31[1]
    c32[:, C_B32A] = b32[0]
    c32[:, C_LT2:C_LT2 + H2] = w21.T
    c32[:, C_LT3:C_LT3 + H2] = w22.T
    c32[:, C_W31:C_W31 + 2] = w31.T
    c32[:, C_W32:C_W32 + 2] = w32.T
    return c32


_PROGRAM_CACHE = {}


def get_program(repeat=1):
    if repeat not in _PROGRAM_CACHE:
        _PROGRAM_CACHE[repeat] = build_program(repeat)
    return _PROGRAM_CACHE[repeat]


def run_on_cores(nc, x_full, c32):
    x_full = np.ascontiguousarray(x_full, dtype=np.float32)
    in_maps = [
        {"x": x_full[c * BS:(c + 1) * BS], "c32": c32}
        for c in range(N_CORES)
    ]
    res = run_bass_kernel_spmd(nc, in_maps, core_ids=list(range(N_CORES)))
    return np.concatenate([res.results[c]["out"] for c in range(N_CORES)], axis=0)


def kernel(x, mean, std, w1, b1, w21, b21, w22, b22, w31, b31, w32, b32, sgn=1,
           **_unused):
    c32 = make_consts(
        np.asarray(mean, np.float32), np.asarray(std, np.float32),
        np.asarray(w1, np.float32), np.asarray(b1, np.float32),
        np.asarray(w21, np.float32), np.asarray(b21, np.float32),
        np.asarray(w22, np.float32), np.asarray(b22, np.float32),
        np.asarray(w31, np.float32), np.asarray(b31, np.float32),
        np.asarray(w32, np.float32), np.asarray(b32, np.float32))
    nc = get_program(repeat=1)
    return run_on_cores(nc, np.asarray(x), c32)


# revision 24
# speedup vs baseline: 1.1568x; 1.1568x over previous
"""v2: instruction-count-minimized kernel for the axon-tunneled trn2 backend.

The execution backend charges ~34us per compute instruction regardless of
size; DMA instructions are cheap. So: fewest, widest compute instructions.

Per core (32768 samples), per super-chunk SC of 4096 samples (all fp32):
  - xt [8, 4096] fm tile loaded by ONE strided gather DMA (col = global row)
  - L1/L2/L3: 8x [128,512]-moving matmuls each -> P [128, 4096] (8 psum banks)
  - bias+relu fused in ONE wide DVE op [128, 4096] per layer
  - L4/L5 heads: 8 matmuls each -> P[0:2, :] and P[32:34, :] (fm)
  - one [34, 4096] PSUM->SBUF copy, two cheap DMAs park heads in DRAM
Epilogue (whole core at once):
  - DMAs gather heads DRAM -> bm32 [32, (1024, 4)] (sample s at
    partition s%32) and x[:, 0:2] -> [32, (1024, 2)]
  - ~20 wide DVE/ACT ops compute the closed-form QP
  - DMAs scatter u back to out[g, 0:2]
"""

import os
import sys

import numpy as np

for _p in ("/opt/trn_rl_repo", os.path.expanduser("~/.axon_site/_ro/trn_rl_repo")):
    if os.path.isdir(_p) and _p not in sys.path:
        sys.path.append(_p)

import concourse.bacc as bacc
import concourse.mybir as mybir
import concourse.tile as tile
from concourse.bass_utils import run_bass_kernel_spmd

dt = mybir.dt
AF = mybir.ActivationFunctionType
ALU = mybir.AluOpType

N_CORES = 8
B_FULL, F, H1, H2, C = 262144, 8, 128, 128, 2
BS = B_FULL // N_CORES    # 32768 per core
P = 128
SC = 4096                 # samples per super-chunk
NSC = BS // SC            # 8
OBS_X, OBS_Y, RADIUS = 40.0, 15.0, 6.0
EPS = 1e-12

# fp32 const block [128, CW32]
C_LT1 = 0      # (w1*std).T in rows 0:8   [8, 128]
C_B1 = 128     # b1 + w1@mean             [128, 1]
C_B21 = 129    # b21                      [128, 1]
C_B22 = 130    # b22                      [128, 1]
C_S0 = 131     # scalar std[0] broadcast  [128, 1]
C_S1 = 132     # std[1]
C_M0 = 133     # mean[0] - OBS_X
C_M1 = 134     # mean[1] - OBS_Y
C_B31A = 135   # b31[0]
C_B31B = 136   # b31[1]
C_B32A = 137   # b32[0]
CW32 = 398
C_LT2 = 138    # w21.T [128, 128]
C_LT3 = 266    # w22.T [128, 128]
C_W31 = 394    # w31.T [128, 2]
C_W32 = 396    # w32.T [128, 2]



def build_program(repeat=1):
    nc = bacc.Bacc("TRN2", target_bir_lowering=False, debug=False,
                   num_devices=N_CORES)
    f32 = dt.float32

    NQ = BS // 32
    x_d = nc.dram_tensor("x", [BS, F], f32, kind="ExternalInput").ap()
    c32_d = nc.dram_tensor("c32", [P, CW32], f32, kind="ExternalInput").ap()
    out_d = nc.dram_tensor("out", [BS, C], f32, kind="ExternalOutput").ap()
    hb_d = nc.dram_tensor("hb", [4, BS], f32).ap()  # heads parked fm in DRAM

    with tile.TileContext(nc) as tc:
        with (
            tc.tile_pool(name="cst", bufs=1) as cstp,
            tc.tile_pool(name="xt", bufs=1) as xtp,
            tc.tile_pool(name="act", bufs=1) as actp,
            tc.tile_pool(name="hd", bufs=1) as hdp,
            tc.tile_pool(name="ep", bufs=1) as epp,
            tc.tile_pool(name="ps", bufs=1, space="PSUM") as psp,
        ):
            c32 = cstp.tile([P, CW32], f32)
            nc.sync.dma_start(c32[:], c32_d[:])

            lt1 = c32[0:F, C_LT1:C_LT1 + H1]
            b1 = c32[:, C_B1:C_B1 + 1]
            b21 = c32[:, C_B21:C_B21 + 1]
            b22 = c32[:, C_B22:C_B22 + 1]
            lt2 = c32[:, C_LT2:C_LT2 + H2]
            lt3 = c32[:, C_LT3:C_LT3 + H2]
            w31 = c32[:, C_W31:C_W31 + 2]
            w32 = c32[:, C_W32:C_W32 + 2]

            for _rep in range(repeat):
                for sc in range(NSC):
                    g0 = sc * SC
                    # fm gather: xt[f, c] = x[g0 + c, f]
                    xt = xtp.tile([F, SC], f32, tag="xt")
                    nc.sync.dma_start(
                        xt[:], x_d[g0:g0 + SC, :].rearrange("g f -> f g"))

                    Ppre = psp.tile([P, SC], f32, tag="P")  # all 8 banks
                    for b in range(8):
                        nc.tensor.matmul(Ppre[:, 512 * b:512 * (b + 1)],
                                         lt1, xt[:, 512 * b:512 * (b + 1)],
                                         start=True, stop=True)
                    h1 = actp.tile([P, SC], f32, tag="h1")
                    nc.vector.tensor_scalar(h1[:], Ppre[:], b1, 0.0,
                                            ALU.add, ALU.max)

                    Ppre2 = psp.tile([P, SC], f32, tag="P")
                    for b in range(8):
                        nc.tensor.matmul(Ppre2[:, 512 * b:512 * (b + 1)],
                                         lt2, h1[:, 512 * b:512 * (b + 1)],
                                         start=True, stop=True)
                    x21 = actp.tile([P, SC], f32, tag="x21")
                    nc.vector.tensor_scalar(x21[:], Ppre2[:], b21, 0.0,
                                            ALU.add, ALU.max)

                    Ppre3 = psp.tile([P, SC], f32, tag="P")
                    for b in range(8):
                        nc.tensor.matmul(Ppre3[:, 512 * b:512 * (b + 1)],
                                         lt3, h1[:, 512 * b:512 * (b + 1)],
                                         start=True, stop=True)
                    x22 = actp.tile([P, SC], f32, tag="x22")
                    nc.vector.tensor_scalar(x22[:], Ppre3[:], b22, 0.0,
                                            ALU.add, ALU.max)

                    PH = psp.tile([P, SC], f32, tag="P")
                    for b in range(8):
                        nc.tensor.matmul(PH[0:2, 512 * b:512 * (b + 1)],
                                         w31, x21[:, 512 * b:512 * (b + 1)],
                                         start=True, stop=True)
                    for b in range(8):
                        nc.tensor.matmul(PH[32:34, 512 * b:512 * (b + 1)],
                                         w32, x22[:, 512 * b:512 * (b + 1)],
                                         start=True, stop=True)
                    hs = hdp.tile([34, SC], f32, tag="hs")
                    nc.vector.tensor_copy(hs[:], PH[0:34, :])
                    nc.sync.dma_start(hb_d[0:2, g0:g0 + SC], hs[0:2, :])
                    nc.sync.dma_start(hb_d[2:4, g0:g0 + SC], hs[32:34, :])

                # ---- epilogue: whole core, batch-major-32 ----
                hbm = epp.tile([32, NQ, 4], f32, tag="hbm")
                # hbm[m, n, k] = hb[k, 32n + m]
                hbv = hb_d.rearrange("k (n m) -> k m n", m=32)
                for k in range(4):
                    nc.sync.dma_start(hbm[:, :, k], hbv[k])
                xb = epp.tile([32, NQ, 2], f32, tag="xb")
                xbv = x_d[:, 0:2].rearrange("(n m) c -> c m n", m=32)
                for k in range(2):
                    nc.sync.dma_start(xb[:, :, k], xbv[k])

                pp = epp.tile([32, NQ, 2], f32, tag="pp")
                d_t = epp.tile([32, NQ, 2], f32, tag="d_t")
                dsq = epp.tile([32, NQ, 2], f32, tag="dsq")
                dp = epp.tile([32, NQ, 2], f32, tag="dp")
                sp0 = epp.tile([32, NQ], f32, tag="sp0")
                sig0 = epp.tile([32, NQ], f32, tag="sig0")
                bar0 = epp.tile([32, NQ], f32, tag="bar0")
                bar = epp.tile([32, NQ], f32, tag="bar")
                v1 = epp.tile([32, NQ], f32, tag="v1")
                hb2 = epp.tile([32, NQ], f32, tag="hb2")
                viol2 = epp.tile([32, NQ], f32, tag="viol2")
                ggq = epp.tile([32, NQ], f32, tag="ggq")
                rec = epp.tile([32, NQ], f32, tag="rec")
                lam2 = epp.tile([32, NQ], f32, tag="lam2")
                u_t = epp.tile([32, NQ, 2], f32, tag="u_t")

                V = nc.vector
                s0c = c32[0:32, C_S0:C_S0 + 1]
                s1c = c32[0:32, C_S1:C_S1 + 1]
                m0c = c32[0:32, C_M0:C_M0 + 1]
                m1c = c32[0:32, C_M1:C_M1 + 1]
                b31a = c32[0:32, C_B31A:C_B31A + 1]
                b31b = c32[0:32, C_B31B:C_B31B + 1]
                b32a = c32[0:32, C_B32A:C_B32A + 1]

                # d = x01 * std01 + (mean01 - obs), fused dual-op
                V.tensor_scalar(d_t[:, :, 0], xb[:, :, 0], s0c, m0c,
                                ALU.mult, ALU.add)
                V.tensor_scalar(d_t[:, :, 1], xb[:, :, 1], s1c, m1c,
                                ALU.mult, ALU.add)
                V.tensor_tensor(dsq[:], d_t[:], d_t[:], ALU.mult)
                # p' = P + b31
                V.tensor_scalar(pp[:, :, 0], hbm[:, :, 0], b31a, None, ALU.add)
                V.tensor_scalar(pp[:, :, 1], hbm[:, :, 1], b31b, None, ALU.add)
                V.tensor_tensor(dp[:], d_t[:], pp[:], ALU.mult)
                # sig0 = sigmoid(S0 + b32[0]) with the bias folded into ACT
                nc.scalar.activation(sig0[:], hbm[:, :, 2], AF.Sigmoid,
                                     bias=b32a)
                V.tensor_tensor(bar0[:], dsq[:, :, 0], dsq[:, :, 1], ALU.add)
                V.tensor_tensor(v1[:], dp[:, :, 0], dp[:, :, 1], ALU.add)
                V.tensor_scalar(bar[:], bar0[:], 2.0, -2.0 * RADIUS * RADIUS,
                                ALU.mult, ALU.add)
                V.tensor_tensor(hb2[:], sig0[:], bar[:], ALU.mult)
                V.tensor_tensor(viol2[:], v1[:], hb2[:], ALU.subtract)
                # lam2 = relu(viol2) / (bar0 + eps/4)
                V.tensor_scalar(viol2[:], viol2[:], 0.0, None, ALU.max)
                V.tensor_scalar(ggq[:], bar0[:], EPS / 4.0, None, ALU.add)
                V.reciprocal(rec[:], ggq[:])
                V.tensor_tensor(lam2[:], viol2[:], rec[:], ALU.mult)
                # u = d * lam2 - p'
                V.tensor_tensor(u_t[:, :, 0], d_t[:, :, 0], lam2[:], ALU.mult)
                V.tensor_tensor(u_t[:, :, 1], d_t[:, :, 1], lam2[:], ALU.mult)
                V.tensor_tensor(u_t[:], u_t[:], pp[:], ALU.subtract)

                outv = out_d.rearrange("(n m) c -> c m n", m=32)
                for k in range(2):
                    nc.sync.dma_start(outv[k], u_t[:, :, k])

    nc.compile()
    return nc


def make_consts(mean, std, w1, b1, w21, b21, w22, b22, w31, b31, w32, b32):
    c32 = np.zeros((P, CW32), dtype=np.float32)
    c32[0:F, C_LT1:C_LT1 + H1] = (w1 * std[None, :]).T
    c32[:, C_B1] = b1 + w1 @ mean
    c32[:, C_B21] = b21
    c32[:, C_B22] = b22
    c32[:, C_S0] = std[0]
    c32[:, C_S1] = std[1]
    c32[:, C_M0] = mean[0] - OBS_X
    c32[:, C_M1] = mean[1] - OBS_Y
    c32[:, C_B31A] = b31[0]
    c32[:, C_B31B] = b31[1]
    c32[:, C_B32A] = b32[0]
    c32[:, C_LT2:C_LT2 + H2] = w21.T
    c32[:, C_LT3:C_LT3 + H2] = w22.T
    c32[:, C_W31:C_W31 + 2] = w31.T
    c32[:, C_W32:C_W32 + 2] = w32.T
    return c32


_PROGRAM_CACHE = {}


def get_program(repeat=1):
    if repeat not in _PROGRAM_CACHE:
        _PROGRAM_CACHE[repeat] = build_program(repeat)
    return _PROGRAM_CACHE[repeat]


def run_on_cores(nc, x_full, c32):
    x_full = np.ascontiguousarray(x_full, dtype=np.float32)
    in_maps = [
        {"x": x_full[c * BS:(c + 1) * BS], "c32": c32}
        for c in range(N_CORES)
    ]
    res = run_bass_kernel_spmd(nc, in_maps, core_ids=list(range(N_CORES)))
    return np.concatenate([res.results[c]["out"] for c in range(N_CORES)], axis=0)


def kernel(x, mean, std, w1, b1, w21, b21, w22, b22, w31, b31, w32, b32, sgn=1,
           **_unused):
    c32 = make_consts(
        np.asarray(mean, np.float32), np.asarray(std, np.float32),
        np.asarray(w1, np.float32), np.asarray(b1, np.float32),
        np.asarray(w21, np.float32), np.asarray(b21, np.float32),
        np.asarray(w22, np.float32), np.asarray(b22, np.float32),
        np.asarray(w31, np.float32), np.asarray(b31, np.float32),
        np.asarray(w32, np.float32), np.asarray(b32, np.float32))
    nc = get_program(repeat=1)
    return run_on_cores(nc, np.asarray(x), c32)
